# revision 4
# baseline (speedup 1.0000x reference)
"""EGNN layer on 8 Trainium2 NeuronCores (Bass/Tile).

Strategy (edge/data parallel per the sharding hint):
  - Sort edges by destination (row) on the host; shard edges across the 8
    cores by destination-node range so each core's aggregation is fully
    local (no cross-core reduction needed).
  - Host marshals per-edge dense streams: h[row], h[col] (bf16), x/geometry
    sidecar streams, selection-matrix indices and masks.  These are pure
    np.take / reshape operations (zero FLOPs) - all arithmetic (both MLPs,
    radial, clipping, segment sums, residuals) runs on device.
  - Device: per 512-edge block, DMA-transpose loads the h-streams as
    [feature x edge] tiles, runs the edge MLP as weight-stationary matmuls
    with SiLU on the scalar engine, computes per-edge coordinate weights,
    and aggregates per-node sums via selection-matrix matmuls into PSUM
    (one PSUM accumulator per 128-node tile).  A final node phase runs the
    node MLP and residual adds.
"""
import math
import numpy as np
import ml_dtypes

import concourse.bacc as bacc
import concourse.bass as bass
import concourse.tile as tile
import concourse.mybir as mybir
from concourse import bass_utils

BF16 = ml_dtypes.bfloat16
N_CORES = 8
CLAMP = 2.0


# ----------------------------------------------------------------------------
# Host-side preparation (marshalling only: sort, pad, np.take, dtype casts)
# ----------------------------------------------------------------------------
def prepare(h, x, edges, we1, be1, we2, be2, wn1, bn1, wn2, bn2, wc1, bc1, wc2,
            n_cores=N_CORES):
    N, D = h.shape
    E = edges.shape[1]
    assert D == 128
    NS = N // n_cores
    assert NS * n_cores == N
    TILES = math.ceil(NS / 128)

    row = np.asarray(edges[0], dtype=np.int64).astype(np.int32)
    col = np.asarray(edges[1], dtype=np.int64).astype(np.int32)
    perm = np.argsort(row, kind="stable")
    rows_s = row[perm]
    cols_s = col[perm]

    # tile boundaries: per core j, tile t covers nodes [j*NS + 128t, j*NS + 128(t+1))
    # edge run for (j,t) found by searchsorted on sorted rows
    tile_lo = np.empty((n_cores, TILES), np.int64)
    tile_hi = np.empty((n_cores, TILES), np.int64)
    for j in range(n_cores):
        for t in range(TILES):
            lo_node = j * NS + t * 128
            hi_node = min(j * NS + (t + 1) * 128, (j + 1) * NS)
            tile_lo[j, t] = np.searchsorted(rows_s, lo_node, "left")
            tile_hi[j, t] = np.searchsorted(rows_s, hi_node, "left")
    run_len = tile_hi - tile_lo
    K_t = np.maximum(np.ceil(run_len / 128).astype(np.int64).max(axis=0), 1)
    # pad total chunk count to a multiple of 4 (512-edge blocks)
    extra = (-K_t.sum()) % 4
    K_t[-1] += extra
    CHUNKS = int(K_t.sum())
    NBLK = CHUNKS // 4
    Ep = CHUNKS * 128

    # chunk -> tile map and start/stop flags (same for all cores)
    chunk_tile = np.repeat(np.arange(TILES), K_t)
    first_chunk = np.zeros(CHUNKS, bool)
    last_chunk = np.zeros(CHUNKS, bool)
    pos = 0
    for t in range(TILES):
        first_chunk[pos] = True
        pos += int(K_t[t])
        last_chunk[pos - 1] = True

    h_bf = np.asarray(h, np.float32).astype(BF16)
    x_f = np.asarray(x, np.float32)
    x_bf = x_f.astype(BF16)

    in_maps = []
    for j in range(n_cores):
        rows_p = np.empty(Ep, np.int32)
        cols_p = np.empty(Ep, np.int32)
        mask_p = np.zeros(Ep, np.float32)
        rowrel = np.zeros(Ep, np.int32)
        pos = 0
        for t in range(TILES):
            lo, hi = tile_lo[j, t], tile_hi[j, t]
            n = int(hi - lo)
            kpad = int(K_t[t]) * 128
            base = j * NS + t * 128
            rows_p[pos:pos + n] = rows_s[lo:hi]
            cols_p[pos:pos + n] = cols_s[lo:hi]
            mask_p[pos:pos + n] = 1.0
            rowrel[pos:pos + n] = rows_s[lo:hi] - base
            rows_p[pos + n:pos + kpad] = base
            cols_p[pos + n:pos + kpad] = 0
            # rowrel stays 0 for pads (payload masked to zero)
            pos += kpad
        assert pos == Ep

        hrow = np.ascontiguousarray(h_bf[rows_p])             # [Ep, 128] bf16
        hcol = np.ascontiguousarray(h_bf[cols_p])
        x8 = np.zeros((Ep, 8), BF16)
        x8[:, 0:3] = x_bf[rows_p]
        x8[:, 3] = BF16(1.0)                 # ones col -> radial bias slot
        x8[:, 4:7] = x_bf[cols_p]

        # block-arranged per-chunk arrays: [NBLK, 128, 4] with [b, p, k] = e(b*512 + k*128 + p)
        def blockify(a):
            return np.ascontiguousarray(
                a.reshape(NBLK, 4, 128).transpose(0, 2, 1))
        rrblk = blockify(rowrel.astype(np.int16))
        mkf = blockify(mask_p.astype(np.float32))
        mkb = blockify(mask_p.astype(BF16))

        sl = slice(j * NS, (j + 1) * NS)
        hs = np.zeros((TILES * 128, 128), np.float32)
        hs[:NS] = np.asarray(h, np.float32)[sl]
        hsT = np.ascontiguousarray(hs.astype(BF16).T)          # [128, TILES*128]
        xres = np.zeros((TILES * 128, 4), np.float32)
        xres[:NS, 0:3] = x_f[sl]

        w1 = np.asarray(we1, np.float32)
        c4 = np.zeros((4, 128), np.float32)
        c4[0:3] = w1[256][None, :].repeat(3, 0) * 0 + w1[256]  # rows 0-2 unused slots
        # C4 rows multiply diff2T rows [d0^2, d1^2, d2^2, 1]:
        c4[0] = w1[256]; c4[1] = w1[256]; c4[2] = w1[256]
        c4[3] = np.asarray(be1, np.float32)

        in_maps.append(dict(
            HROW=hrow, HCOL=hcol, X8=x8.reshape(Ep // 512, 4, 128, 8)
                .transpose(0, 2, 1, 3).copy(),
            RRB=rrblk, MKF=mkf, MKB=mkb,
            HST=hsT, HRES=hs, XRES=xres,
            WA=np.ascontiguousarray(w1[:128].astype(BF16)),
            WB=np.ascontiguousarray(w1[128:256].astype(BF16)),
            C4=c4.astype(BF16),
            WE2=np.asarray(we2, np.float32).astype(BF16),
            WC1=np.asarray(wc1, np.float32).astype(BF16),
            WC2=np.asarray(wc2, np.float32).astype(BF16),
            BE2=np.asarray(be2, np.float32).reshape(128, 1),
            BC1=np.asarray(bc1, np.float32).reshape(128, 1),
            AN1=np.asarray(wn1, np.float32)[:128].astype(BF16),
            BN1=np.asarray(wn1, np.float32)[128:].astype(BF16),
            WN2=np.asarray(wn2, np.float32).astype(BF16),
            BN1C=np.asarray(bn1, np.float32).reshape(128, 1),
            BN2R=np.asarray(bn2, np.float32).astype(BF16).reshape(1, 128),
        ))

    meta = dict(N=N, E=E, NS=NS, TILES=TILES, NBLK=NBLK, CHUNKS=CHUNKS, Ep=Ep,
                chunk_tile=chunk_tile, first_chunk=first_chunk,
                last_chunk=last_chunk, n_cores=n_cores)
    return in_maps, meta


# ----------------------------------------------------------------------------
# Device program
# ----------------------------------------------------------------------------
def build_nc(meta):
    NBLK, TILES, Ep = meta["NBLK"], meta["TILES"], meta["Ep"]
    chunk_tile = meta["chunk_tile"]
    first_chunk = meta["first_chunk"]
    last_chunk = meta["last_chunk"]
    NT128 = TILES * 128

    nc = bacc.Bacc("TRN2", target_bir_lowering=False, debug=False)
    dt = mybir.dt
    f32, bf, i16, u8 = dt.float32, dt.bfloat16, dt.int16, dt.uint8

    HROW = nc.dram_tensor("HROW", (Ep, 128), bf, kind="ExternalInput")
    HCOL = nc.dram_tensor("HCOL", (Ep, 128), bf, kind="ExternalInput")
    X8 = nc.dram_tensor("X8", (NBLK, 128, 4, 8), bf, kind="ExternalInput")
    RRB = nc.dram_tensor("RRB", (NBLK, 128, 4), i16, kind="ExternalInput")
    MKF = nc.dram_tensor("MKF", (NBLK, 128, 4), f32, kind="ExternalInput")
    MKB = nc.dram_tensor("MKB", (NBLK, 128, 4), bf, kind="ExternalInput")
    HST = nc.dram_tensor("HST", (128, NT128), bf, kind="ExternalInput")
    HRES = nc.dram_tensor("HRES", (NT128, 128), f32, kind="ExternalInput")
    XRES = nc.dram_tensor("XRES", (NT128, 4), f32, kind="ExternalInput")
    WA = nc.dram_tensor("WA", (128, 128), bf, kind="ExternalInput")
    WB = nc.dram_tensor("WB", (128, 128), bf, kind="ExternalInput")
    C4 = nc.dram_tensor("C4", (4, 128), bf, kind="ExternalInput")
    WE2 = nc.dram_tensor("WE2", (128, 128), bf, kind="ExternalInput")
    WC1 = nc.dram_tensor("WC1", (128, 128), bf, kind="ExternalInput")
    WC2 = nc.dram_tensor("WC2", (128, 1), bf, kind="ExternalInput")
    BE2 = nc.dram_tensor("BE2", (128, 1), f32, kind="ExternalInput")
    BC1 = nc.dram_tensor("BC1", (128, 1), f32, kind="ExternalInput")
    AN1 = nc.dram_tensor("AN1", (128, 128), bf, kind="ExternalInput")
    BN1 = nc.dram_tensor("BN1", (128, 128), bf, kind="ExternalInput")
    WN2 = nc.dram_tensor("WN2", (128, 128), bf, kind="ExternalInput")
    BN1C = nc.dram_tensor("BN1C", (128, 1), f32, kind="ExternalInput")
    BN2R = nc.dram_tensor("BN2R", (1, 128), bf, kind="ExternalInput")
    HOUT = nc.dram_tensor("HOUT", (NT128, 128), f32, kind="ExternalOutput")
    XOUT = nc.dram_tensor("XOUT", (NT128, 4), f32, kind="ExternalOutput")

    from concourse.masks import make_identity
    Silu = mybir.ActivationFunctionType.Silu
    Alu = mybir.AluOpType

    with tile.TileContext(nc) as tc:
        with tc.tile_pool(name="const", bufs=1) as cp:
            # constants
            wa = cp.tile([128, 128], bf, tag="wa")
            nc.sync.dma_start(out=wa[:], in_=WA.ap())
            wb = cp.tile([128, 128], bf, tag="wb")
            nc.sync.dma_start(out=wb[:], in_=WB.ap())
            c4 = cp.tile([4, 128], bf, tag="c4")
            nc.sync.dma_start(out=c4[:], in_=C4.ap())
            we2 = cp.tile([128, 128], bf, tag="we2")
            nc.sync.dma_start(out=we2[:], in_=WE2.ap())
            wc1 = cp.tile([128, 128], bf, tag="wc1")
            nc.sync.dma_start(out=wc1[:], in_=WC1.ap())
            wc2 = cp.tile([128, 1], bf, tag="wc2")
            nc.sync.dma_start(out=wc2[:], in_=WC2.ap())
            be2 = cp.tile([128, 1], f32, tag="be2")
            nc.sync.dma_start(out=be2[:], in_=BE2.ap())
            bc1 = cp.tile([128, 1], f32, tag="bc1")
            nc.sync.dma_start(out=bc1[:], in_=BC1.ap())
            ident = cp.tile([128, 128], bf, tag="ident")
            make_identity(nc, ident[:])
            iota = cp.tile([128, 128], i16, tag="iota")
            nc.gpsimd.iota(iota[:], pattern=[[1, 128]], base=0,
                           channel_multiplier=0)
            aggTm = cp.tile([128, NT128], bf, tag="aggTm")

            with tc.tile_pool(name="stream", bufs=3) as sp, \
                 tc.tile_pool(name="smallin", bufs=3) as si, \
                 tc.tile_pool(name="work", bufs=2) as wk, \
                 tc.tile_pool(name="ps_big", bufs=2, space="PSUM") as psb, \
                 tc.tile_pool(name="ps_mt", bufs=2, space="PSUM") as psm, \
                 tc.tile_pool(name="ps_small", bufs=1, space="PSUM") as pss, \
                 tc.tile_pool(name="ps_agg", bufs=2, space="PSUM") as psa, \
                 tc.tile_pool(name="outp", bufs=2) as op:

                aggp = None
                for b in range(NBLK):
                    hrT = sp.tile([128, 512], bf, tag="hrT")
                    nc.sync.dma_start(out=hrT[:], in_=HROW.ap()[b * 512:(b + 1) * 512, :],
                                      transpose=True)
                    hcT = sp.tile([128, 512], bf, tag="hcT")
                    nc.sync.dma_start(out=hcT[:], in_=HCOL.ap()[b * 512:(b + 1) * 512, :],
                                      transpose=True)
                    x8 = si.tile([128, 4, 8], bf, tag="x8")
                    nc.sync.dma_start(out=x8[:], in_=X8.ap()[b])
                    rr = si.tile([128, 4], i16, tag="rr")
                    nc.sync.dma_start(out=rr[:], in_=RRB.ap()[b])
                    mkf = si.tile([128, 4], f32, tag="mkf")
                    nc.sync.dma_start(out=mkf[:], in_=MKF.ap()[b])
                    mkb = si.tile([128, 4], bf, tag="mkb")
                    nc.sync.dma_start(out=mkb[:], in_=MKB.ap()[b])

                    # geometry in [e x c]: diff (col 3 = 1 - 0 = 1), diff^2
                    dec = wk.tile([128, 4, 4], bf, tag="dec")
                    nc.vector.tensor_tensor(out=dec[:], in0=x8[:, :, 0:4],
                                            in1=x8[:, :, 4:8], op=Alu.subtract)
                    d2e = wk.tile([128, 4, 4], bf, tag="d2e")
                    nc.vector.tensor_tensor(out=d2e[:], in0=dec[:],
                                            in1=dec[:], op=Alu.mult)
                    d2tp = pss.tile([4, 512], bf, tag="d2tp")
                    for k in range(4):
                        nc.tensor.matmul(d2tp[:, k * 128:(k + 1) * 128],
                                         lhsT=d2e[:, k, :], rhs=ident[:],
                                         is_transpose=True, start=True, stop=True)
                    d2s = wk.tile([4, 512], bf, tag="d2s")
                    nc.vector.tensor_copy(out=d2s[:], in_=d2tp[:])

                    # m1 = A.T@hrow + B.T@hcol + C4.T@d2T  (radial*c + be1)
                    m1 = psb.tile([128, 512], f32, tag="big")
                    nc.tensor.matmul(m1[:], lhsT=wa[:], rhs=hrT[:], start=True, stop=False)
                    nc.tensor.matmul(m1[:], lhsT=wb[:], rhs=hcT[:], start=False, stop=False)
                    nc.tensor.matmul(m1[:], lhsT=c4[:], rhs=d2s[:], start=False, stop=True)
                    s1 = wk.tile([128, 512], bf, tag="s1")
                    nc.scalar.activation(out=s1[:], in_=m1[:], func=Silu)

                    mp = psb.tile([128, 512], f32, tag="big")
                    nc.tensor.matmul(mp[:], lhsT=we2[:], rhs=s1[:], start=True, stop=True)
                    mfe = wk.tile([128, 512], bf, tag="mfe")
                    nc.scalar.activation(out=mfe[:], in_=mp[:], func=Silu, bias=be2[:])

                    c1p = psb.tile([128, 512], f32, tag="big")
                    nc.tensor.matmul(c1p[:], lhsT=wc1[:], rhs=mfe[:], start=True, stop=True)
                    c1 = wk.tile([128, 512], bf, tag="c1")
                    nc.scalar.activation(out=c1[:], in_=c1p[:], func=Silu, bias=bc1[:])

                    # w per chunk: [128e x 1] = c1_chunk.T @ wc2
                    wp = pss.tile([128, 4], f32, tag="wp")
                    for k in range(4):
                        nc.tensor.matmul(wp[:, k:k + 1], lhsT=c1[:, k * 128:(k + 1) * 128],
                                         rhs=wc2[:], start=True, stop=True)
                    wsb = wk.tile([128, 4], f32, tag="wsb")
                    nc.vector.tensor_tensor(out=wsb[:], in0=wp[:], in1=mkf[:], op=Alu.mult)

                    # mT: transpose m_fe chunks -> [e x f] (bf16 psum)
                    mtp = psm.tile([128, 512], bf, tag="mtp")
                    for k in range(4):
                        nc.tensor.matmul(mtp[:, k * 128:(k + 1) * 128],
                                         lhsT=mfe[:, k * 128:(k + 1) * 128],
                                         rhs=ident[:], is_transpose=True,
                                         start=True, stop=True)
                    # payload [128, 4, 132]: [m(128) | trans(3) | mask(1)]
                    pay = wk.tile([128, 4, 132], bf, tag="pay")
                    nc.vector.tensor_tensor(
                        out=pay[:, :, 0:128],
                        in0=mtp[:].rearrange("p (k f) -> p k f", k=4),
                        in1=mkb[:, :, None].to_broadcast([128, 4, 128]),
                        op=Alu.mult)
                    # trans = clip(diff * w) ; w already masked
                    tr = wk.tile([128, 4, 3], f32, tag="tr")
                    nc.vector.tensor_tensor(
                        out=tr[:],
                        in0=dec[:, :, 0:3],
                        in1=wsb[:, :, None].to_broadcast([128, 4, 3]),
                        op=Alu.mult)
                    tr2 = wk.tile([128, 4, 3], f32, tag="tr2")
                    nc.vector.tensor_scalar(out=tr2[:], in0=tr[:], scalar1=CLAMP,
                                            scalar2=None, op0=Alu.min)
                    nc.vector.tensor_scalar(out=pay[:, :, 128:131], in0=tr2[:],
                                            scalar1=-CLAMP, scalar2=None, op0=Alu.max)
                    nc.vector.tensor_copy(out=pay[:, :, 131:132], in_=mkb[:, :, None])

                    # Sel [128e x 4 x 128n]
                    sel = wk.tile([128, 4, 128], bf, tag="sel")
                    nc.vector.tensor_tensor(
                        out=sel[:],
                        in0=iota[:, None, :].to_broadcast([128, 4, 128]),
                        in1=rr[:, :, None].to_broadcast([128, 4, 128]),
                        op=Alu.is_equal)

                    # aggregation per chunk
                    for k in range(4):
                        ci = b * 4 + k
                        t = int(chunk_tile[ci])
                        if first_chunk[ci]:
                            aggp = psa.tile([128, 132], f32, tag="agg")
                        nc.tensor.matmul(aggp[:], lhsT=sel[:, k, :], rhs=pay[:, k, :],
                                         start=bool(first_chunk[ci]),
                                         stop=bool(last_chunk[ci]))
                        if last_chunk[ci]:
                            # evacuate tile t
                            agg_sb = op.tile([128, 132], f32, tag="agg_sb")
                            nc.vector.tensor_copy(out=agg_sb[:], in_=aggp[:])
                            agg_bf = op.tile([128, 128], bf, tag="agg_bf")
                            nc.vector.tensor_copy(out=agg_bf[:], in_=agg_sb[:, 0:128])
                            atp = psm.tile([128, 128], bf, tag="mtp")
                            nc.tensor.matmul(atp[:], lhsT=agg_bf[:], rhs=ident[:],
                                             is_transpose=True, start=True, stop=True)
                            nc.vector.tensor_copy(out=aggTm[:, t * 128:(t + 1) * 128],
                                                  in_=atp[:])
                            # x_out for tile t
                            xres_t = op.tile([128, 4], f32, tag="xres_t")
                            nc.sync.dma_start(out=xres_t[:],
                                              in_=XRES.ap()[t * 128:(t + 1) * 128, :])
                            cnt = op.tile([128, 1], f32, tag="cnt")
                            nc.vector.tensor_scalar(out=cnt[:], in0=agg_sb[:, 131:132],
                                                    scalar1=1.0, scalar2=None, op0=Alu.max)
                            rcp = op.tile([128, 1], f32, tag="rcp")
                            nc.vector.reciprocal(out=rcp[:], in_=cnt[:])
                            xo = op.tile([128, 4], f32, tag="xo")
                            nc.vector.tensor_scalar(out=xo[:], in0=agg_sb[:, 128:132],
                                                    scalar1=rcp[:], scalar2=None,
                                                    op0=Alu.mult)
                            nc.vector.tensor_tensor(out=xo[:], in0=xo[:], in1=xres_t[:],
                                                    op=Alu.add)
                            nc.sync.dma_start(out=XOUT.ap()[t * 128:(t + 1) * 128, :],
                                              in_=xo[:])

            # ---------------- node phase ----------------
            with tc.tile_pool(name="np_in", bufs=2) as npi, \
                 tc.tile_pool(name="np_ps", bufs=2, space="PSUM") as npp, \
                 tc.tile_pool(name="np_out", bufs=2) as npo:
                an1 = cp.tile([128, 128], bf, tag="an1")
                nc.sync.dma_start(out=an1[:], in_=AN1.ap())
                bn1t = cp.tile([128, 128], bf, tag="bn1t")
                nc.sync.dma_start(out=bn1t[:], in_=BN1.ap())
                wn2 = cp.tile([128, 128], bf, tag="wn2")
                nc.sync.dma_start(out=wn2[:], in_=WN2.ap())
                bn1c = cp.tile([128, 1], f32, tag="bn1c")
                nc.sync.dma_start(out=bn1c[:], in_=BN1C.ap())
                bn2r = cp.tile([1, 128], bf, tag="bn2r")
                nc.sync.dma_start(out=bn2r[:], in_=BN2R.ap())
                ones1 = cp.tile([1, 128], bf, tag="ones1")
                nc.vector.memset(ones1[:], 1.0)
                hst = cp.tile([128, NT128], bf, tag="hst")
                nc.sync.dma_start(out=hst[:], in_=HST.ap())

                NGRP = math.ceil(NT128 / 512)
                for g in range(NGRP):
                    n0 = g * 512
                    n1 = min(n0 + 512, NT128)
                    nn = n1 - n0
                    z1 = npp.tile([128, 512], f32, tag="z1")
                    nc.tensor.matmul(z1[:, 0:nn], lhsT=an1[:], rhs=hst[:, n0:n1],
                                     start=True, stop=False)
                    nc.tensor.matmul(z1[:, 0:nn], lhsT=bn1t[:], rhs=aggTm[:, n0:n1],
                                     start=False, stop=True)
                    sz = npi.tile([128, 512], bf, tag="sz")
                    nc.scalar.activation(out=sz[:, 0:nn], in_=z1[:, 0:nn], func=Silu,
                                         bias=bn1c[:])
                    for k in range(nn // 128):
                        z2 = npp.tile([128, 128], f32, tag="z2")
                        nc.tensor.matmul(z2[:], lhsT=sz[:, k * 128:(k + 1) * 128],
                                         rhs=wn2[:], start=True, stop=False)
                        nc.tensor.matmul(z2[:], lhsT=ones1[:], rhs=bn2r[:], start=False, stop=True)
                        hres = npi.tile([128, 128], f32, tag="hres")
                        r0 = n0 + k * 128
                        nc.sync.dma_start(out=hres[:], in_=HRES.ap()[r0:r0 + 128, :])
                        ho = npo.tile([128, 128], f32, tag="ho")
                        nc.vector.tensor_tensor(out=ho[:], in0=z2[:], in1=hres[:],
                                                op=Alu.add)
                        nc.sync.dma_start(out=HOUT.ap()[r0:r0 + 128, :], in_=ho[:])

    nc.compile()
    return nc


# ----------------------------------------------------------------------------
# Entry point
# ----------------------------------------------------------------------------
_CACHE = {}


def kernel(**inputs):
    in_maps, meta = prepare(**inputs)
    key = (meta["N"], meta["E"], meta["NBLK"], meta["CHUNKS"])
    if key not in _CACHE:
        _CACHE[key] = build_nc(meta)
    nc = _CACHE[key]
    res = bass_utils.run_bass_kernel_spmd(
        nc, in_maps, core_ids=list(range(meta["n_cores"])))
    return assemble(res.results, meta)


def assemble(results, meta):
    N, NS = meta["N"], meta["NS"]
    h_out = np.empty((N, 128), np.float32)
    x_out = np.empty((N, 3), np.float32)
    for j in range(meta["n_cores"]):
        h_out[j * NS:(j + 1) * NS] = results[j]["HOUT"][:NS]
        x_out[j * NS:(j + 1) * NS] = results[j]["XOUT"][:NS, 0:3]
    return h_out, x_out


# revision 6
# speedup vs baseline: 1.7710x; 1.7710x over previous
"""EGNN layer on 8 Trainium2 NeuronCores (Bass/Tile).

Strategy (edge/data parallel per the sharding hint):
  - Sort edges by destination (row) on the host; shard edges across the 8
    cores by destination-node range so each core's aggregation is fully
    local (no cross-core reduction needed).
  - Host marshals per-edge dense streams: h[row], h[col] (bf16), x/geometry
    sidecar streams, selection-matrix indices and masks.  These are pure
    np.take / reshape operations (zero FLOPs) - all arithmetic (both MLPs,
    radial, clipping, segment sums, residuals) runs on device.
  - Device: per 512-edge block, DMA-transpose loads the h-streams as
    [feature x edge] tiles, runs the edge MLP as weight-stationary matmuls
    with SiLU on the scalar engine, computes per-edge coordinate weights,
    and aggregates per-node sums via selection-matrix matmuls into PSUM
    (one PSUM accumulator per 128-node tile).  A final node phase runs the
    node MLP and residual adds.
"""
import math
import numpy as np
import ml_dtypes

import concourse.bacc as bacc
import concourse.bass as bass
import concourse.tile as tile
import concourse.mybir as mybir
from concourse import bass_utils

BF16 = ml_dtypes.bfloat16
N_CORES = 8
CLAMP = 2.0


# ----------------------------------------------------------------------------
# Host-side preparation (marshalling only: sort, pad, np.take, dtype casts)
# ----------------------------------------------------------------------------
def prepare(h, x, edges, we1, be1, we2, be2, wn1, bn1, wn2, bn2, wc1, bc1, wc2,
            n_cores=N_CORES):
    N, D = h.shape
    E = edges.shape[1]
    assert D == 128
    NS = N // n_cores
    assert NS * n_cores == N
    TILES = math.ceil(NS / 128)

    row = np.asarray(edges[0], dtype=np.int64).astype(np.int32)
    col = np.asarray(edges[1], dtype=np.int64).astype(np.int32)
    perm = np.argsort(row, kind="stable")
    rows_s = row[perm]
    cols_s = col[perm]

    # tile boundaries: per core j, tile t covers nodes [j*NS + 128t, j*NS + 128(t+1))
    # edge run for (j,t) found by searchsorted on sorted rows
    tile_lo = np.empty((n_cores, TILES), np.int64)
    tile_hi = np.empty((n_cores, TILES), np.int64)
    for j in range(n_cores):
        for t in range(TILES):
            lo_node = j * NS + t * 128
            hi_node = min(j * NS + (t + 1) * 128, (j + 1) * NS)
            tile_lo[j, t] = np.searchsorted(rows_s, lo_node, "left")
            tile_hi[j, t] = np.searchsorted(rows_s, hi_node, "left")
    run_len = tile_hi - tile_lo
    K_t = np.maximum(np.ceil(run_len / 128).astype(np.int64).max(axis=0), 1)
    # pad total chunk count to a multiple of 4 (512-edge blocks)
    extra = (-K_t.sum()) % 4
    K_t[-1] += extra
    CHUNKS = int(K_t.sum())
    NBLK = CHUNKS // 4
    Ep = CHUNKS * 128

    # chunk -> tile map and start/stop flags (same for all cores)
    chunk_tile = np.repeat(np.arange(TILES), K_t)
    first_chunk = np.zeros(CHUNKS, bool)
    last_chunk = np.zeros(CHUNKS, bool)
    pos = 0
    for t in range(TILES):
        first_chunk[pos] = True
        pos += int(K_t[t])
        last_chunk[pos - 1] = True

    h_bf = np.asarray(h, np.float32).astype(BF16)
    x_f = np.asarray(x, np.float32)
    x_bf = x_f.astype(BF16)

    in_maps = []
    for j in range(n_cores):
        rows_p = np.empty(Ep, np.int32)
        cols_p = np.empty(Ep, np.int32)
        mask_p = np.zeros(Ep, np.float32)
        rowrel = np.zeros(Ep, np.int32)
        pos = 0
        for t in range(TILES):
            lo, hi = tile_lo[j, t], tile_hi[j, t]
            n = int(hi - lo)
            kpad = int(K_t[t]) * 128
            base = j * NS + t * 128
            rows_p[pos:pos + n] = rows_s[lo:hi]
            cols_p[pos:pos + n] = cols_s[lo:hi]
            mask_p[pos:pos + n] = 1.0
            rowrel[pos:pos + n] = rows_s[lo:hi] - base
            rows_p[pos + n:pos + kpad] = base
            cols_p[pos + n:pos + kpad] = 0
            # rowrel stays 0 for pads (payload masked to zero)
            pos += kpad
        assert pos == Ep

        hrow = h_bf[rows_p].reshape(NBLK, 512, 128).transpose(0, 2, 1)
        hrow = np.ascontiguousarray(hrow)                     # [NBLK,128,512] f x e
        hcol = h_bf[cols_p].reshape(NBLK, 512, 128).transpose(0, 2, 1)
        hcol = np.ascontiguousarray(hcol)
        x8 = np.zeros((Ep, 8), BF16)
        x8[:, 0:3] = x_bf[rows_p]
        x8[:, 3] = BF16(1.0)                 # ones col -> radial bias slot
        x8[:, 4:7] = x_bf[cols_p]

        # block-arranged per-chunk arrays: [NBLK, 128, 4] with [b, p, k] = e(b*512 + k*128 + p)
        def blockify(a):
            return np.ascontiguousarray(
                a.reshape(NBLK, 4, 128).transpose(0, 2, 1))
        rrblk = blockify(rowrel.astype(np.int16))
        mkf = blockify(mask_p.astype(np.float32))
        mkb = blockify(mask_p.astype(BF16))
        x8b = np.ascontiguousarray(x8.reshape(NBLK, 4, 128, 8).transpose(0, 2, 1, 3))
        misc = np.empty((NBLK, 128, 96), np.uint8)
        misc[:, :, 0:64] = x8b.reshape(NBLK, 128, 32).view(np.uint8)
        misc[:, :, 64:72] = rrblk.view(np.uint8)
        misc[:, :, 72:88] = mkf.view(np.uint8)
        misc[:, :, 88:96] = mkb.view(np.uint8)

        sl = slice(j * NS, (j + 1) * NS)
        hs = np.zeros((TILES * 128, 128), np.float32)
        hs[:NS] = np.asarray(h, np.float32)[sl]
        hsT = np.ascontiguousarray(hs.astype(BF16).T)          # [128, TILES*128]
        xres = np.zeros((TILES * 128, 4), np.float32)
        xres[:NS, 0:3] = x_f[sl]

        w1 = np.asarray(we1, np.float32)
        c4 = np.zeros((4, 128), np.float32)
        c4[0:3] = w1[256][None, :].repeat(3, 0) * 0 + w1[256]  # rows 0-2 unused slots
        # C4 rows multiply diff2T rows [d0^2, d1^2, d2^2, 1]:
        c4[0] = w1[256]; c4[1] = w1[256]; c4[2] = w1[256]
        c4[3] = np.asarray(be1, np.float32)

        in_maps.append(dict(
            HROW=hrow, HCOL=hcol, MISC=misc,
            HST=hsT, HRES=hs, XRES=xres,
            WA=np.ascontiguousarray(w1[:128].astype(BF16)),
            WB=np.ascontiguousarray(w1[128:256].astype(BF16)),
            C4=c4.astype(BF16),
            WE2=np.asarray(we2, np.float32).astype(BF16),
            WC1=np.asarray(wc1, np.float32).astype(BF16),
            WC2=np.asarray(wc2, np.float32).astype(BF16),
            BE2=np.asarray(be2, np.float32).reshape(128, 1),
            BC1=np.asarray(bc1, np.float32).reshape(128, 1),
            AN1=np.asarray(wn1, np.float32)[:128].astype(BF16),
            BN1=np.asarray(wn1, np.float32)[128:].astype(BF16),
            WN2=np.asarray(wn2, np.float32).astype(BF16),
            BN1C=np.asarray(bn1, np.float32).reshape(128, 1),
            BN2R=np.asarray(bn2, np.float32).astype(BF16).reshape(1, 128),
        ))

    meta = dict(N=N, E=E, NS=NS, TILES=TILES, NBLK=NBLK, CHUNKS=CHUNKS, Ep=Ep,
                chunk_tile=chunk_tile, first_chunk=first_chunk,
                last_chunk=last_chunk, n_cores=n_cores)
    return in_maps, meta


# ----------------------------------------------------------------------------
# Device program
# ----------------------------------------------------------------------------
def build_nc(meta):
    NBLK, TILES, Ep = meta["NBLK"], meta["TILES"], meta["Ep"]
    chunk_tile = meta["chunk_tile"]
    first_chunk = meta["first_chunk"]
    last_chunk = meta["last_chunk"]
    NT128 = TILES * 128

    nc = bacc.Bacc("TRN2", target_bir_lowering=False, debug=False)
    dt = mybir.dt
    f32, bf, i16, u8 = dt.float32, dt.bfloat16, dt.int16, dt.uint8

    HROW = nc.dram_tensor("HROW", (NBLK, 128, 512), bf, kind="ExternalInput")
    HCOL = nc.dram_tensor("HCOL", (NBLK, 128, 512), bf, kind="ExternalInput")
    MISC = nc.dram_tensor("MISC", (NBLK, 128, 96), u8, kind="ExternalInput")
    HST = nc.dram_tensor("HST", (128, NT128), bf, kind="ExternalInput")
    HRES = nc.dram_tensor("HRES", (NT128, 128), f32, kind="ExternalInput")
    XRES = nc.dram_tensor("XRES", (NT128, 4), f32, kind="ExternalInput")
    WA = nc.dram_tensor("WA", (128, 128), bf, kind="ExternalInput")
    WB = nc.dram_tensor("WB", (128, 128), bf, kind="ExternalInput")
    C4 = nc.dram_tensor("C4", (4, 128), bf, kind="ExternalInput")
    WE2 = nc.dram_tensor("WE2", (128, 128), bf, kind="ExternalInput")
    WC1 = nc.dram_tensor("WC1", (128, 128), bf, kind="ExternalInput")
    WC2 = nc.dram_tensor("WC2", (128, 1), bf, kind="ExternalInput")
    BE2 = nc.dram_tensor("BE2", (128, 1), f32, kind="ExternalInput")
    BC1 = nc.dram_tensor("BC1", (128, 1), f32, kind="ExternalInput")
    AN1 = nc.dram_tensor("AN1", (128, 128), bf, kind="ExternalInput")
    BN1 = nc.dram_tensor("BN1", (128, 128), bf, kind="ExternalInput")
    WN2 = nc.dram_tensor("WN2", (128, 128), bf, kind="ExternalInput")
    BN1C = nc.dram_tensor("BN1C", (128, 1), f32, kind="ExternalInput")
    BN2R = nc.dram_tensor("BN2R", (1, 128), bf, kind="ExternalInput")
    HOUT = nc.dram_tensor("HOUT", (NT128, 128), f32, kind="ExternalOutput")
    XOUT = nc.dram_tensor("XOUT", (NT128, 4), f32, kind="ExternalOutput")

    from concourse.masks import make_identity
    Silu = mybir.ActivationFunctionType.Silu
    Alu = mybir.AluOpType

    with tile.TileContext(nc) as tc:
        with tc.tile_pool(name="const", bufs=1) as cp:
            # constants
            wa = cp.tile([128, 128], bf, tag="wa")
            nc.sync.dma_start(out=wa[:], in_=WA.ap())
            wb = cp.tile([128, 128], bf, tag="wb")
            nc.sync.dma_start(out=wb[:], in_=WB.ap())
            c4 = cp.tile([4, 128], bf, tag="c4")
            nc.sync.dma_start(out=c4[:], in_=C4.ap())
            we2 = cp.tile([128, 128], bf, tag="we2")
            nc.sync.dma_start(out=we2[:], in_=WE2.ap())
            wc1 = cp.tile([128, 128], bf, tag="wc1")
            nc.sync.dma_start(out=wc1[:], in_=WC1.ap())
            wc2 = cp.tile([128, 1], bf, tag="wc2")
            nc.sync.dma_start(out=wc2[:], in_=WC2.ap())
            be2 = cp.tile([128, 1], f32, tag="be2")
            nc.sync.dma_start(out=be2[:], in_=BE2.ap())
            bc1 = cp.tile([128, 1], f32, tag="bc1")
            nc.sync.dma_start(out=bc1[:], in_=BC1.ap())
            ident = cp.tile([128, 128], bf, tag="ident")
            make_identity(nc, ident[:])
            iota = cp.tile([128, 128], i16, tag="iota")
            nc.gpsimd.iota(iota[:], pattern=[[1, 128]], base=0,
                           channel_multiplier=0)
            aggTm = cp.tile([128, NT128], bf, tag="aggTm")

            with tc.tile_pool(name="stream", bufs=4) as sp, \
                 tc.tile_pool(name="smallin", bufs=3) as si, \
                 tc.tile_pool(name="work", bufs=2) as wk, \
                 tc.tile_pool(name="ps_big", bufs=2, space="PSUM") as psb, \
                 tc.tile_pool(name="ps_mt", bufs=2, space="PSUM") as psm, \
                 tc.tile_pool(name="ps_small", bufs=1, space="PSUM") as pss, \
                 tc.tile_pool(name="ps_agg", bufs=2, space="PSUM") as psa, \
                 tc.tile_pool(name="outp", bufs=2) as op:

                aggp = None
                for b in range(NBLK):
                    hrT = sp.tile([128, 512], bf, tag="hrT")
                    nc.sync.dma_start(out=hrT[:], in_=HROW.ap()[b])
                    hcT = sp.tile([128, 512], bf, tag="hcT")
                    nc.sync.dma_start(out=hcT[:], in_=HCOL.ap()[b])
                    misc = si.tile([128, 96], u8, tag="misc")
                    nc.scalar.dma_start(out=misc[:], in_=MISC.ap()[b])
                    x8 = misc[:, 0:64].bitcast(bf).rearrange("p (k c) -> p k c", k=4)
                    rr = misc[:, 64:72].bitcast(i16)
                    mkf = misc[:, 72:88].bitcast(f32)
                    mkb = misc[:, 88:96].bitcast(bf)

                    # geometry in [e x c]: diff (col 3 = 1 - 0 = 1), diff^2
                    dec = wk.tile([128, 4, 4], bf, tag="dec")
                    nc.vector.tensor_tensor(out=dec[:], in0=x8[:, :, 0:4],
                                            in1=x8[:, :, 4:8], op=Alu.subtract)
                    d2e = wk.tile([128, 4, 4], bf, tag="d2e")
                    nc.vector.tensor_tensor(out=d2e[:], in0=dec[:],
                                            in1=dec[:], op=Alu.mult)
                    d2tp = pss.tile([4, 512], bf, tag="d2tp")
                    for k in range(4):
                        nc.tensor.matmul(d2tp[:, k * 128:(k + 1) * 128],
                                         lhsT=d2e[:, k, :], rhs=ident[:],
                                         is_transpose=True, start=True, stop=True)
                    d2s = wk.tile([4, 512], bf, tag="d2s")
                    nc.vector.tensor_copy(out=d2s[:], in_=d2tp[:])

                    # m1 = A.T@hrow + B.T@hcol + C4.T@d2T  (radial*c + be1)
                    m1 = psb.tile([128, 512], f32, tag="big")
                    nc.tensor.matmul(m1[:], lhsT=wa[:], rhs=hrT[:], start=True, stop=False)
                    nc.tensor.matmul(m1[:], lhsT=wb[:], rhs=hcT[:], start=False, stop=False)
                    nc.tensor.matmul(m1[:], lhsT=c4[:], rhs=d2s[:], start=False, stop=True)
                    s1 = wk.tile([128, 512], bf, tag="s1")
                    nc.scalar.activation(out=s1[:], in_=m1[:], func=Silu)

                    mp = psb.tile([128, 512], f32, tag="big")
                    nc.tensor.matmul(mp[:], lhsT=we2[:], rhs=s1[:], start=True, stop=True)
                    mfe = wk.tile([128, 512], bf, tag="mfe")
                    nc.scalar.activation(out=mfe[:], in_=mp[:], func=Silu, bias=be2[:])

                    c1p = psb.tile([128, 512], f32, tag="big")
                    nc.tensor.matmul(c1p[:], lhsT=wc1[:], rhs=mfe[:], start=True, stop=True)
                    c1 = wk.tile([128, 512], bf, tag="c1")
                    nc.scalar.activation(out=c1[:], in_=c1p[:], func=Silu, bias=bc1[:])

                    # w per chunk: [128e x 1] = c1_chunk.T @ wc2
                    wp = pss.tile([128, 4], f32, tag="wp")
                    for k in range(4):
                        nc.tensor.matmul(wp[:, k:k + 1], lhsT=c1[:, k * 128:(k + 1) * 128],
                                         rhs=wc2[:], start=True, stop=True)
                    wsb = wk.tile([128, 4], f32, tag="wsb")
                    nc.vector.tensor_tensor(out=wsb[:], in0=wp[:], in1=mkf, op=Alu.mult)

                    # mT: transpose m_fe chunks -> [e x f] (bf16 psum)
                    mtp = psm.tile([128, 512], bf, tag="mtp")
                    for k in range(4):
                        nc.tensor.matmul(mtp[:, k * 128:(k + 1) * 128],
                                         lhsT=mfe[:, k * 128:(k + 1) * 128],
                                         rhs=ident[:], is_transpose=True,
                                         start=True, stop=True)
                    # payload [128, 4, 132]: [m(128) | trans(3) | mask(1)]
                    pay = wk.tile([128, 4, 132], bf, tag="pay")
                    nc.vector.tensor_tensor(
                        out=pay[:, :, 0:128],
                        in0=mtp[:].rearrange("p (k f) -> p k f", k=4),
                        in1=mkb[:, :, None].to_broadcast([128, 4, 128]),
                        op=Alu.mult)
                    # trans = clip(diff * w) ; w already masked
                    tr = wk.tile([128, 4, 3], f32, tag="tr")
                    nc.vector.tensor_tensor(
                        out=tr[:],
                        in0=dec[:, :, 0:3],
                        in1=wsb[:, :, None].to_broadcast([128, 4, 3]),
                        op=Alu.mult)
                    tr2 = wk.tile([128, 4, 3], f32, tag="tr2")
                    nc.vector.tensor_scalar(out=tr2[:], in0=tr[:], scalar1=CLAMP,
                                            scalar2=None, op0=Alu.min)
                    nc.vector.tensor_scalar(out=pay[:, :, 128:131], in0=tr2[:],
                                            scalar1=-CLAMP, scalar2=None, op0=Alu.max)
                    nc.vector.tensor_copy(out=pay[:, :, 131:132], in_=mkb[:, :, None])

                    # Sel [128e x 4 x 128n]
                    sel = wk.tile([128, 4, 128], bf, tag="sel")
                    nc.vector.tensor_tensor(
                        out=sel[:],
                        in0=iota[:, None, :].to_broadcast([128, 4, 128]),
                        in1=rr[:, :, None].to_broadcast([128, 4, 128]),
                        op=Alu.is_equal)

                    # aggregation per chunk
                    for k in range(4):
                        ci = b * 4 + k
                        t = int(chunk_tile[ci])
                        if first_chunk[ci]:
                            aggp = psa.tile([128, 132], f32, tag="agg")
                        nc.tensor.matmul(aggp[:], lhsT=sel[:, k, :], rhs=pay[:, k, :],
                                         start=bool(first_chunk[ci]),
                                         stop=bool(last_chunk[ci]))
                        if last_chunk[ci]:
                            # evacuate tile t
                            agg_sb = op.tile([128, 132], f32, tag="agg_sb")
                            nc.vector.tensor_copy(out=agg_sb[:], in_=aggp[:])
                            agg_bf = op.tile([128, 128], bf, tag="agg_bf")
                            nc.vector.tensor_copy(out=agg_bf[:], in_=agg_sb[:, 0:128])
                            atp = psm.tile([128, 128], bf, tag="mtp")
                            nc.tensor.matmul(atp[:], lhsT=agg_bf[:], rhs=ident[:],
                                             is_transpose=True, start=True, stop=True)
                            nc.vector.tensor_copy(out=aggTm[:, t * 128:(t + 1) * 128],
                                                  in_=atp[:])
                            # x_out for tile t
                            xres_t = op.tile([128, 4], f32, tag="xres_t")
                            nc.scalar.dma_start(out=xres_t[:],
                                                in_=XRES.ap()[t * 128:(t + 1) * 128, :])
                            cnt = op.tile([128, 1], f32, tag="cnt")
                            nc.vector.tensor_scalar(out=cnt[:], in0=agg_sb[:, 131:132],
                                                    scalar1=1.0, scalar2=None, op0=Alu.max)
                            rcp = op.tile([128, 1], f32, tag="rcp")
                            nc.vector.reciprocal(out=rcp[:], in_=cnt[:])
                            xo = op.tile([128, 4], f32, tag="xo")
                            nc.vector.tensor_scalar(out=xo[:], in0=agg_sb[:, 128:132],
                                                    scalar1=rcp[:], scalar2=None,
                                                    op0=Alu.mult)
                            nc.vector.tensor_tensor(out=xo[:], in0=xo[:], in1=xres_t[:],
                                                    op=Alu.add)
                            nc.scalar.dma_start(out=XOUT.ap()[t * 128:(t + 1) * 128, :],
                                                in_=xo[:])

            # ---------------- node phase ----------------
            with tc.tile_pool(name="np_in", bufs=2) as npi, \
                 tc.tile_pool(name="np_ps", bufs=2, space="PSUM") as npp, \
                 tc.tile_pool(name="np_out", bufs=2) as npo:
                an1 = cp.tile([128, 128], bf, tag="an1")
                nc.sync.dma_start(out=an1[:], in_=AN1.ap())
                bn1t = cp.tile([128, 128], bf, tag="bn1t")
                nc.sync.dma_start(out=bn1t[:], in_=BN1.ap())
                wn2 = cp.tile([128, 128], bf, tag="wn2")
                nc.sync.dma_start(out=wn2[:], in_=WN2.ap())
                bn1c = cp.tile([128, 1], f32, tag="bn1c")
                nc.sync.dma_start(out=bn1c[:], in_=BN1C.ap())
                bn2r = cp.tile([1, 128], bf, tag="bn2r")
                nc.sync.dma_start(out=bn2r[:], in_=BN2R.ap())
                ones1 = cp.tile([1, 128], bf, tag="ones1")
                nc.vector.memset(ones1[:], 1.0)
                hst = cp.tile([128, NT128], bf, tag="hst")
                nc.sync.dma_start(out=hst[:], in_=HST.ap())

                NGRP = math.ceil(NT128 / 512)
                for g in range(NGRP):
                    n0 = g * 512
                    n1 = min(n0 + 512, NT128)
                    nn = n1 - n0
                    z1 = npp.tile([128, 512], f32, tag="z1")
                    nc.tensor.matmul(z1[:, 0:nn], lhsT=an1[:], rhs=hst[:, n0:n1],
                                     start=True, stop=False)
                    nc.tensor.matmul(z1[:, 0:nn], lhsT=bn1t[:], rhs=aggTm[:, n0:n1],
                                     start=False, stop=True)
                    sz = npi.tile([128, 512], bf, tag="sz")
                    nc.scalar.activation(out=sz[:, 0:nn], in_=z1[:, 0:nn], func=Silu,
                                         bias=bn1c[:])
                    for k in range(nn // 128):
                        z2 = npp.tile([128, 128], f32, tag="z2")
                        nc.tensor.matmul(z2[:], lhsT=sz[:, k * 128:(k + 1) * 128],
                                         rhs=wn2[:], start=True, stop=False)
                        nc.tensor.matmul(z2[:], lhsT=ones1[:], rhs=bn2r[:], start=False, stop=True)
                        hres = npi.tile([128, 128], f32, tag="hres")
                        r0 = n0 + k * 128
                        nc.scalar.dma_start(out=hres[:], in_=HRES.ap()[r0:r0 + 128, :])
                        ho = npo.tile([128, 128], f32, tag="ho")
                        nc.vector.tensor_tensor(out=ho[:], in0=z2[:], in1=hres[:],
                                                op=Alu.add)
                        nc.scalar.dma_start(out=HOUT.ap()[r0:r0 + 128, :], in_=ho[:])

    nc.compile()
    return nc


# ----------------------------------------------------------------------------
# Entry point
# ----------------------------------------------------------------------------
_CACHE = {}


def kernel(**inputs):
    in_maps, meta = prepare(**inputs)
    key = (meta["N"], meta["E"], meta["NBLK"], meta["CHUNKS"])
    if key not in _CACHE:
        _CACHE[key] = build_nc(meta)
    nc = _CACHE[key]
    res = bass_utils.run_bass_kernel_spmd(
        nc, in_maps, core_ids=list(range(meta["n_cores"])))
    return assemble(res.results, meta)


def assemble(results, meta):
    N, NS = meta["N"], meta["NS"]
    h_out = np.empty((N, 128), np.float32)
    x_out = np.empty((N, 3), np.float32)
    for j in range(meta["n_cores"]):
        h_out[j * NS:(j + 1) * NS] = results[j]["HOUT"][:NS]
        x_out[j * NS:(j + 1) * NS] = results[j]["XOUT"][:NS, 0:3]
    return h_out, x_out
